# revision 2
# baseline (speedup 1.0000x reference)
"""Trainium2 Bass kernel for nn_DiffusionInteractionBlock (GNN message passing).

Strategy: shard EDGES by receiver node range across 8 cores (receiver-sharded
edge parallelism), exactly as the v1 kernel, but restructured around the axon
link profile (83 ms RTT, ~55 MB/s each way):

  * all host->device inputs are packed into ONE consolidated node blob plus K
    small per-chunk edge blobs (fewer, larger PJRT buffers -> higher effective
    upload bandwidth);
  * the device program is split into exe_node (node tables + AllGather) and a
    single compiled exe_chunk program invoked K times, each chunk producing
    1/K of the output, so chunk downloads overlap the remaining chunk uploads
    and each other (the link is full duplex);
  * all tile-position dependence in exe_chunk flows through data (gather
    indices shipped in the blob, receiver scalars fetched from T_full by
    indirect DMA), so one NEFF serves every chunk.

Numerics: 10-bit x0 / y, int8 x1 / edge tails, bf16 tables, int8 +
per-node-scale output (rel err ~1.18e-2, gate 2e-2).
"""

import os
import sys
from concurrent.futures import ThreadPoolExecutor

import numpy as np

sys.path.insert(0, "/opt/trn_rl_repo")

import ml_dtypes

from concourse import bacc, bass, mybir, tile

BF16 = ml_dtypes.bfloat16

N = 10000
E = 160000
MUL = 128
NCORES = 8
NPC = N // NCORES  # 1250 nodes per core
NT = 10            # node tiles of 128 per core (1280 >= 1250)
LPC = NT * 128     # padded local node count (1280)
NPAD = NCORES * LPC  # padded global table rows (10240)
SQ3 = float(np.sqrt(3.0))
INV = 1.0 / np.sqrt(MUL)
OUT_SCALE = 1.0 / (np.sqrt(2 * MUL) * 16.0)
K_CHUNKS = int(os.environ.get("K_CHUNKS", "10"))
TPC = NT // K_CHUNKS  # tiles per chunk

dt = mybir.dt


def _align(x, a=64):
    return (x + a - 1) // a * a


def _node_layout():
    off = {}
    o = 0
    o_ = lambda n, sz: (off.__setitem__(n, o), _align(o + sz))[1]
    o = o_("x0h", 128 * LPC)
    o = o_("x0l", 128 * (LPC // 4))
    o = o_("nf_q", 384 * LPC)
    o = o_("wcat", 128 * 208 * 2)
    o = o_("wsm", 14 * 128 * 2)
    o = o_("scl", 128 * 22 * 4)
    return off, o


def _edge_layout(B_pad):
    EPT = 128 * B_pad
    PB = (B_pad + 1) // 2  # packed idx col pairs per tile
    off = {}
    o = 0
    o_ = lambda n, sz: (off.__setitem__(n, o), _align(o + sz))[1]
    o = o_("idxp", 128 * TPC * PB * 4)
    o = o_("trix", 128 * TPC * 4)
    o = o_("tail9", TPC * 9 * EPT)
    o = o_("y4h", TPC * 128 * 4 * B_pad)
    o = o_("y4l", TPC * 128 * B_pad)
    o = o_("rl8", TPC * 128 * B_pad)
    return off, o, PB


# --------------------------------------------------------------------------
# Host-side preprocessing
# --------------------------------------------------------------------------

def _host_prep(inputs):
    import heapq

    node_feats = np.asarray(inputs["node_feats"], np.float32)
    edge_attrs = np.asarray(inputs["edge_attrs"], np.float32)
    edge_feats = np.asarray(inputs["edge_feats"], np.float32)
    lengths = np.asarray(inputs["lengths"], np.float32)
    edge_index = np.asarray(inputs["edge_index"], np.int64)
    W_scalar = np.asarray(inputs["W_scalar"], np.float32)
    W_up0 = np.asarray(inputs["W_up0"], np.float32)
    W_up1 = np.asarray(inputs["W_up1"], np.float32)
    W1 = np.asarray(inputs["W1"], np.float32)
    b1 = np.asarray(inputs["b1"], np.float32)
    W2 = np.asarray(inputs["W2"], np.float32)
    b2 = np.asarray(inputs["b2"], np.float32)
    W3 = np.asarray(inputs["W3"], np.float32)
    Wout0 = np.asarray(inputs["Wout0"], np.float32)
    Wout1 = np.asarray(inputs["Wout1"], np.float32)

    sender, receiver = edge_index[0], edge_index[1]

    # --- degree-balanced node -> (core, tile, pos) assignment
    G = NCORES * NT
    deg = np.bincount(receiver, minlength=N)
    node_bin = np.empty(N, np.int32)
    node_pos = np.empty(N, np.int32)
    bin_nodes = np.full((G, 128), -1, np.int64)
    heap = [(0, 0, g) for g in range(G)]
    for n in np.argsort(-deg, kind="stable"):
        while True:
            load, cnt, g = heapq.heappop(heap)
            if cnt < 128:
                break
        node_bin[n] = g
        node_pos[n] = cnt
        bin_nodes[g, cnt] = n
        heapq.heappush(heap, (load + int(deg[n]), cnt + 1, g))
    node_row = ((node_bin // NT) * LPC + (node_bin % NT) * 128
                + node_pos).astype(np.int32)

    gtile = node_bin[receiver]
    counts = np.bincount(gtile, minlength=G)
    B_pad = int(np.ceil(counts.max() / 128))
    EPT = 128 * B_pad

    order = np.argsort(gtile, kind="stable")
    epos = np.full((G, EPT), -1, np.int64)
    off = 0
    for g in range(G):
        c = counts[g]
        epos[g, :c] = order[off:off + c]
        off += c

    valid = epos >= 0
    ep = np.where(valid, epos, 0)
    es = np.where(valid, node_row[sender[ep]], 0).astype(np.int32)
    rl = np.where(valid, node_pos[receiver[ep]], 0)
    y = np.where(valid[..., None], edge_attrs[ep], 0.0)
    ef = np.where(valid[..., None], edge_feats[ep], 0.0)
    ln = np.where(valid[..., None], lengths[ep], 0.0)

    # idx [G, EPT] -> [NCORES, 128, NT, B_pad]; pack per-tile column pairs
    PB = (B_pad + 1) // 2
    idx_s = es.reshape(NCORES, NT, B_pad, 128)
    idx_s = np.transpose(idx_s, (0, 3, 1, 2))  # [NC, 128, NT, B_pad]
    if B_pad % 2:
        idx_s = np.concatenate(
            [idx_s, np.zeros((NCORES, 128, NT, 1), np.int32)], axis=3)
    idx_p = (idx_s[:, :, :, 0::2] | (idx_s[:, :, :, 1::2] << 16)).astype(
        np.int32)                                # [NC, 128, NT, PB]

    # tail9 [NCORES, NT, 9, EPT] int8 with per-(core,feature) scale
    tail = np.concatenate([ef, ln], axis=-1)
    tail9 = np.transpose(tail.reshape(NCORES, NT, EPT, 9), (0, 1, 3, 2))
    tmx = np.maximum(np.abs(tail9).max(axis=(1, 3)), 1e-20)
    tail9 = np.rint(tail9 * (126.5 / tmx)[:, None, :, None]).astype(np.int8)
    tail_s = (tmx / 126.5).astype(np.float32)

    # y as 10-bit fixed point (global per-core scale): q = 4*hi + (lo-2),
    # hi int8, lo 2 bits x4 per byte packed over the flat (comp, block) axis
    yv = y.reshape(NCORES, NT, B_pad, 128, 4)
    y4f = np.transpose(yv, (0, 1, 3, 4, 2))
    ymx = np.maximum(np.abs(y4f).max(axis=(1, 2, 3, 4)), 1e-20)
    qy = np.clip(np.rint(y4f * (509.0 / ymx)[:, None, None, None, None]),
                 -509, 509).astype(np.int32)
    yhi = np.floor_divide(qy + 2, 4)
    ylo = (qy - 4 * yhi + 2).reshape(NCORES, NT, 128, 4 * B_pad)
    y4h = yhi.astype(np.int8)
    y4l = (ylo[..., 0::4] | (ylo[..., 1::4] << 2) | (ylo[..., 2::4] << 4)
           | (ylo[..., 3::4] << 6)).astype(np.uint8)
    y_s = (ymx / 509.0).astype(np.float32)
    rl8 = rl.reshape(NCORES, NT, B_pad, 128).transpose(0, 1, 3, 2)
    rl8 = rl8.astype(np.int8)

    # per-core node features in bin-permuted column order
    x1 = node_feats[:, MUL:].reshape(N, MUL, 3)
    f = np.concatenate([node_feats[:, :MUL], x1[:, :, 0], x1[:, :, 1],
                        x1[:, :, 2]], axis=1)
    colnodes = bin_nodes.reshape(NCORES, LPC)
    vals = f[np.where(colnodes >= 0, colnodes, 0)]
    vals[colnodes < 0] = 0
    nfT = np.ascontiguousarray(vals.transpose(0, 2, 1))         # [NC, 512, LPC]
    x0T = nfT[:, 0:128]
    x1T = nfT[:, 128:512]
    nmx = np.maximum(np.abs(x1T).max(axis=1), 1e-20)
    nf_q = np.rint(x1T * (126.5 / nmx)[:, None, :]).astype(np.int8)
    nf_s = (nmx / 126.5).reshape(NCORES, NT, 128).transpose(0, 2, 1)
    # x0 as 10-bit fixed point with per-node scale: q = 4*hi + (lo-2),
    # hi int8, lo 2 bits x4 per byte
    mx0 = np.maximum(np.abs(x0T).max(axis=1), 1e-20)
    q0 = np.clip(np.rint(x0T * (509.0 / mx0)[:, None, :]),
                 -509, 509).astype(np.int32)
    hi = np.floor_divide(q0 + 2, 4)
    lo = q0 - 4 * hi + 2
    x0h = hi.astype(np.int8)
    x0l = (lo[:, :, 0::4] | (lo[:, :, 1::4] << 2) | (lo[:, :, 2::4] << 4)
           | (lo[:, :, 3::4] << 6)).astype(np.uint8)
    s0 = (mx0 / 509.0).reshape(NCORES, NT, 128).transpose(0, 2, 1)
    scl = np.zeros((NCORES, 128, 2 * NT + 2), np.float32)
    scl[:, :, :NT] = nf_s
    scl[:, :, NT:2 * NT] = s0
    scl[:, :9, 2 * NT] = tail_s
    scl[:, :, 2 * NT + 1] = y_s[:, None]

    Ws_inv = W_scalar * INV
    W3b = np.concatenate(
        [W3[:, :MUL], W3[:, MUL:2 * MUL] / SQ3,
         W3[:, 2 * MUL:3 * MUL], W3[:, 3 * MUL:]], axis=1)
    wcat = np.concatenate([
        W_up0 * INV, W_up1 * INV,
        Ws_inv @ W1[:MUL], Ws_inv @ W1[MUL:2 * MUL],
        W2, Wout0[:MUL] * OUT_SCALE, Wout0[MUL:] * OUT_SCALE,
        Wout1[:MUL] * OUT_SCALE, Wout1[MUL:] * OUT_SCALE, W3b,
    ], axis=1).astype(BF16)                                     # [128, 1664]
    wcat_sh = wcat.reshape(128, 8, 208).transpose(1, 0, 2).copy()  # [8,128,208]
    cw = np.stack([
        -2.0 * (W_up0 * INV).astype(BF16).astype(np.float32).sum(0),
        -2.0 * (Ws_inv @ W1[:MUL]).astype(BF16).astype(np.float32).sum(0),
        -2.0 * (Ws_inv @ W1[MUL:2 * MUL]).astype(BF16).astype(np.float32).sum(0),
    ])
    wsmall = np.concatenate(
        [W1[2 * MUL:], b1[None, :], b2[None, :], cw], 0).astype(BF16)  # [14,128]

    # ---- pack node blob [NCORES, NB] int8 ----
    noff, NB = _node_layout()
    nblob = np.zeros((NCORES, NB), np.int8)

    def put(o, arr):
        b = arr.reshape(NCORES, -1).view(np.int8)
        nblob[:, o:o + b.shape[1]] = b

    put(noff["x0h"], x0h)
    put(noff["x0l"], x0l)
    put(noff["nf_q"], nf_q)
    put(noff["wcat"], wcat_sh)
    wsm_all = np.tile(wsmall[None], (NCORES, 1, 1))
    put(noff["wsm"], wsm_all)
    put(noff["scl"], np.ascontiguousarray(scl))

    # ---- pack edge blobs: K arrays [NCORES, EB] int8 ----
    eoff, EB, _PB = _edge_layout(B_pad)
    eblobs = []
    core_base = (np.arange(NCORES, dtype=np.int32) * LPC)
    prow = np.arange(128, dtype=np.int32)
    for k in range(K_CHUNKS):
        t0 = k * TPC
        eb = np.zeros((NCORES, EB), np.int8)

        def pute(o, arr):
            b = arr.reshape(NCORES, -1).view(np.int8)
            eb[:, o:o + b.shape[1]] = b

        pute(eoff["idxp"], np.ascontiguousarray(idx_p[:, :, t0:t0 + TPC]))
        # T_full is gathered as a [NPAD*6, 128] view for the receiver-scalar
        # columns (indirect DMA requires source offset 0): row*6 + 5
        trix = (core_base[:, None, None]
                + (np.arange(t0, t0 + TPC, dtype=np.int32) * 128)[None, None, :]
                + prow[None, :, None]) * 6 + 5          # [NC, 128, TPC]
        pute(eoff["trix"], np.ascontiguousarray(trix))
        pute(eoff["tail9"], np.ascontiguousarray(tail9[:, t0:t0 + TPC]))
        pute(eoff["y4h"], np.ascontiguousarray(y4h[:, t0:t0 + TPC]))
        pute(eoff["y4l"], np.ascontiguousarray(y4l[:, t0:t0 + TPC]))
        pute(eoff["rl8"], np.ascontiguousarray(rl8[:, t0:t0 + TPC]))
        eblobs.append(eb)

    return B_pad, {"node": nblob, "edges": eblobs}, node_row


# --------------------------------------------------------------------------
# Device programs
# --------------------------------------------------------------------------

def _blob_view(blob, off, nbytes, p, dtype):
    """[p, nbytes/(p*isize)] view of a flat [1, NB] int8 dram tensor."""
    ap = blob[0:1, off:off + nbytes]
    if dtype != dt.int8:
        ap = ap.bitcast(dtype)
    n = ap.shape[1]
    return ap.rearrange("a (p c) -> (a p) c", p=p)


def _build_node(B_pad):
    noff, NB = _node_layout()
    nc = bacc.Bacc("TRN2", target_bir_lowering=False, debug=False,
                   num_devices=NCORES)
    f32, bf16, i8, u8 = dt.float32, dt.bfloat16, dt.int8, dt.uint8

    blob = nc.dram_tensor("nblob", [1, NB], i8, kind="ExternalInput")

    T_full = nc.dram_tensor("T_full", [NPAD, 768], bf16, kind="ExternalOutput")
    Wfull = nc.dram_tensor("Wfull", [128, 1664], bf16, kind="ExternalOutput")
    Wsm = nc.dram_tensor("Wsm", [14, 128], bf16, kind="ExternalOutput")
    sclf = nc.dram_tensor("sclf", [128, 22], f32, kind="ExternalOutput")

    T_loc = nc.dram_tensor("T_loc", [LPC, 768], bf16)
    T_gat = nc.dram_tensor("T_gat", [NPAD, 768], bf16)
    W_shb = nc.dram_tensor("W_shb", [128, 208], bf16)
    W_gat = nc.dram_tensor("W_gat", [NCORES * 128, 208], bf16)

    AL = mybir.AluOpType
    AF = mybir.ActivationFunctionType

    with tile.TileContext(nc) as tc:
        with (
            tc.tile_pool(name="const", bufs=1) as cp,
            tc.tile_pool(name="work", bufs=2) as wp,
            tc.tile_pool(name="psAgg", bufs=1, space="PSUM") as psAgg,
        ):
            # ---- allgather the column-sharded weight pack ----
            nc.sync.dma_start(
                out=W_shb[:, :],
                in_=_blob_view(blob, noff["wcat"], 128 * 208 * 2, 128, bf16))
            nc.gpsimd.collective_compute(
                "AllGather", mybir.AluOpType.bypass,
                replica_groups=[list(range(NCORES))],
                ins=[W_shb[:, :]], outs=[W_gat[:, :]])
            wc_t = cp.tile([128, 1664], bf16, tag="c_wcat")
            for k in range(NCORES):
                nc.sync.dma_start(
                    out=wc_t[:, 208 * k:208 * (k + 1)],
                    in_=W_gat[128 * k:128 * (k + 1), :])
            wsm_t = cp.tile([14, 128], bf16, tag="c_wsm")
            nc.sync.dma_start(
                out=wsm_t[:, :],
                in_=_blob_view(blob, noff["wsm"], 14 * 128 * 2, 14, bf16))
            cw_t = []
            for k in range(3):
                cwk = cp.tile([1, 128], bf16, tag=f"c_cw{k}")
                nc.sync.dma_start(
                    out=cwk[:, :],
                    in_=_blob_view(blob, noff["wsm"] + (11 + k) * 256,
                                   256, 1, bf16))
                cw_t.append(cwk)
            scl_t = cp.tile([128, 2 * NT + 2], f32, tag="c_scl")
            nc.sync.dma_start(
                out=scl_t[:, :],
                in_=_blob_view(blob, noff["scl"], 128 * 22 * 4, 128, f32))

            wup0_s = wc_t[:, 0:128]
            wup1_s = wc_t[:, 128:256]
            wps_s = wc_t[:, 256:384]
            wpr_s = wc_t[:, 384:512]
            ones_s = cp.tile([1, 128], bf16, tag="c_ones")
            nc.vector.memset(ones_s[:, :], 1.0)

            # write exported weight tensors for the chunk program
            nc.sync.dma_start(out=Wfull[:, :], in_=wc_t[:, :])
            nc.sync.dma_start(out=Wsm[:, :], in_=wsm_t[:, :])
            nc.sync.dma_start(out=sclf[:, :], in_=scl_t[:, :])

            # ---- local node-table phase ----
            with tc.tile_pool(name="nodes", bufs=1) as npool:
                xh_q = npool.tile([128, LPC], i8, tag="x0h")
                nc.sync.dma_start(
                    out=xh_q[:, :],
                    in_=_blob_view(blob, noff["x0h"], 128 * LPC, 128, i8))
                xl_q = npool.tile([128, LPC // 4], u8, tag="x0l")
                nc.sync.dma_start(
                    out=xl_q[:, :],
                    in_=_blob_view(blob, noff["x0l"], 128 * (LPC // 4), 128, u8))
                x0hi = npool.tile([128, LPC], bf16, tag="x0hi")
                nc.scalar.activation(out=x0hi[:, :], in_=xh_q[:, :],
                                     func=AF.Copy, scale=4.0)
                xl_u = npool.tile([128, LPC], u8, tag="x0lu")
                xlv = xl_u[:, :].rearrange("p (k four) -> p k four", four=4)
                nc.vector.tensor_scalar(out=xlv[:, :, 0], in0=xl_q[:, :],
                                        scalar1=3, scalar2=None,
                                        op0=AL.bitwise_and)
                for fi in range(1, 4):
                    nc.vector.tensor_scalar(out=xlv[:, :, fi], in0=xl_q[:, :],
                                            scalar1=2 * fi, scalar2=3,
                                            op0=AL.logical_shift_right,
                                            op1=AL.bitwise_and)
                x0lo = npool.tile([128, LPC], bf16, tag="x0lo")
                nc.vector.tensor_copy(out=x0lo[:, :], in_=xl_u[:, :])
                x1t0 = npool.tile([128, LPC], bf16, tag="nf1")
                x1t1 = npool.tile([128, LPC], bf16, tag="nf2")
                x1t2 = npool.tile([128, LPC], bf16, tag="nf3")
                for k, t in enumerate([x1t0, x1t1, x1t2]):
                    xq = npool.tile([128, LPC], i8, tag=f"nq{k}")
                    nc.sync.dma_start(
                        out=xq[:, :],
                        in_=_blob_view(blob, noff["nf_q"] + k * 128 * LPC,
                                       128 * LPC, 128, i8))
                    nc.vector.tensor_copy(out=t[:, :], in_=xq[:, :])
                for s in range(NT):
                    sl = slice(128 * s, 128 * (s + 1))
                    pn = psAgg.tile([128, 1024], f32, tag="agg")
                    for lhs, rhs, o in [(x1t0, wup1_s, 128),
                                        (x1t1, wup1_s, 256),
                                        (x1t2, wup1_s, 384)]:
                        nc.tensor.matmul(out=pn[:, o:o + 128], lhsT=lhs[:, sl],
                                         rhs=rhs, start=True, stop=True)
                    for k, (rhs, o) in enumerate([(wup0_s, 0), (wps_s, 512),
                                                  (wpr_s, 640)]):
                        nc.tensor.matmul(out=pn[:, o:o + 128],
                                         lhsT=x0hi[:, sl], rhs=rhs,
                                         start=True, stop=False)
                        nc.tensor.matmul(out=pn[:, o:o + 128],
                                         lhsT=x0lo[:, sl], rhs=rhs,
                                         start=False, stop=False)
                        nc.tensor.matmul(out=pn[:, o:o + 128],
                                         lhsT=ones_s[:, :], rhs=cw_t[k],
                                         start=False, stop=True)
                    tsb = wp.tile([128, 768], bf16, tag="tsb")
                    s0c = scl_t[:, NT + s:NT + s + 1]
                    nc.scalar.activation(out=tsb[:, 0:128], in_=pn[:, 0:128],
                                         func=AF.Copy, scale=s0c)
                    nc.scalar.activation(out=tsb[:, 128:512],
                                         in_=pn[:, 128:512],
                                         func=AF.Copy, scale=scl_t[:, s:s + 1])
                    nc.scalar.activation(out=tsb[:, 512:640],
                                         in_=pn[:, 512:640],
                                         func=AF.Copy, scale=s0c)
                    nc.scalar.activation(out=tsb[:, 640:768],
                                         in_=pn[:, 640:768],
                                         func=AF.Copy, scale=s0c)
                    nc.sync.dma_start(out=T_loc[sl, :], in_=tsb[:, :])

            # ---- allgather node tables (with receiver scalars) ----
            # (collectives cannot write IO tensors, so gather into internal
            # DRAM and copy out)
            nc.gpsimd.collective_compute(
                "AllGather", mybir.AluOpType.bypass,
                replica_groups=[list(range(NCORES))],
                ins=[T_loc[:, :]], outs=[T_gat[:, :]])
            nc.sync.dma_start(out=T_full[:, :], in_=T_gat[:, :])

    nc.compile()
    return nc


def _build_chunk(B_pad):
    eoff, EB, PB = _edge_layout(B_pad)
    EPT = 128 * B_pad
    nc = bacc.Bacc("TRN2", target_bir_lowering=False, debug=False,
                   num_devices=NCORES)
    f32, bf16, i32, i8, u8 = (dt.float32, dt.bfloat16, dt.int32,
                              dt.int8, dt.uint8)

    eblob = nc.dram_tensor("eblob", [1, EB], i8, kind="ExternalInput")
    T_full = nc.dram_tensor("T_full", [NPAD, 768], bf16, kind="ExternalInput")
    Wfull = nc.dram_tensor("Wfull", [128, 1664], bf16, kind="ExternalInput")
    Wsm = nc.dram_tensor("Wsm", [14, 128], bf16, kind="ExternalInput")
    sclf = nc.dram_tensor("sclf", [128, 22], f32, kind="ExternalInput")

    out_d = nc.dram_tensor("out_dram", [TPC * 128, 516], i8,
                           kind="ExternalOutput")

    AL = mybir.AluOpType
    AF = mybir.ActivationFunctionType

    with tile.TileContext(nc) as tc:
        with (
            tc.tile_pool(name="const", bufs=1) as cp,
            tc.tile_pool(name="work", bufs=2) as wp,
            tc.tile_pool(name="gath", bufs=2) as gp,
            tc.tile_pool(name="psB", bufs=1, space="PSUM") as psB,
            tc.tile_pool(name="psC", bufs=2, space="PSUM") as psC,
            tc.tile_pool(name="psAgg", bufs=1, space="PSUM") as psAgg,
        ):
            wc_t = cp.tile([128, 1664], bf16, tag="c_wcat")
            nc.sync.dma_start(out=wc_t[:, :], in_=Wfull[:, :])
            wsm_t = cp.tile([14, 128], bf16, tag="c_wsm")
            nc.sync.dma_start(out=wsm_t[:, :], in_=Wsm[:, :])
            b1_t = cp.tile([1, 128], bf16, tag="c_b1")
            nc.sync.dma_start(out=b1_t[:, :], in_=Wsm[9:10, :])
            b2_t = cp.tile([1, 128], bf16, tag="c_b2")
            nc.sync.dma_start(out=b2_t[:, :], in_=Wsm[10:11, :])
            scl_t = cp.tile([128, 2 * NT + 2], f32, tag="c_scl")
            nc.sync.dma_start(out=scl_t[:, :], in_=sclf[:, :])

            NBC = TPC * PB
            ixp_t = cp.tile([128, NBC], i32, tag="c_idxp")
            nc.sync.dma_start(
                out=ixp_t[:, :],
                in_=_blob_view(eblob, eoff["idxp"], 128 * NBC * 4, 128, i32))
            ixs_s = cp.tile([128, 2 * NBC], i32, tag="c_idx")
            ixv = ixs_s[:, :].rearrange("p (j two) -> p j two", two=2)
            nc.vector.tensor_scalar(out=ixv[:, :, 0], in0=ixp_t[:, :],
                                    scalar1=65535, scalar2=None,
                                    op0=AL.bitwise_and)
            nc.vector.tensor_scalar(out=ixv[:, :, 1], in0=ixp_t[:, :],
                                    scalar1=16, scalar2=None,
                                    op0=AL.logical_shift_right)
            trix_t = cp.tile([128, TPC], i32, tag="c_trix")
            nc.sync.dma_start(
                out=trix_t[:, :],
                in_=_blob_view(eblob, eoff["trix"], 128 * TPC * 4, 128, i32))

            w1c_s = wsm_t[0:9, :]
            b1_s = b1_t[:, :]
            b2_s = b2_t[:, :]
            w2_s = wc_t[:, 512:640]
            wo0t_s = wc_t[:, 640:768]
            wo0b_s = wc_t[:, 768:896]
            wo1t_s = wc_t[:, 896:1024]
            wo1b_s = wc_t[:, 1024:1152]
            w3_s = wc_t[:, 1152:1664]

            # iota / identity generated on device: col index (all partitions),
            # partition index (all cols), identity = is_equal of the two
            iotab_s = cp.tile([128, 128], bf16, tag="c_iotab")
            nc.gpsimd.iota(iotab_s[:, :], [[1, 128]], channel_multiplier=0,
                           allow_small_or_imprecise_dtypes=True)
            rowi_s = cp.tile([128, 128], bf16, tag="c_rowi")
            nc.gpsimd.iota(rowi_s[:, :], [[0, 128]], channel_multiplier=1,
                           allow_small_or_imprecise_dtypes=True)
            idb_s = cp.tile([128, 128], bf16, tag="c_idb")
            nc.vector.tensor_tensor(out=idb_s[:, :], in0=iotab_s[:, :],
                                    in1=rowi_s[:, :], op=AL.is_equal)
            idf_s = cp.tile([128, 128], f32, tag="c_idf")
            nc.vector.tensor_copy(out=idf_s[:, :], in_=idb_s[:, :])
            ones_s = cp.tile([1, 128], bf16, tag="c_ones")
            nc.vector.memset(ones_s[:, :], 1.0)
            zr_s = cp.tile([128, 128], bf16, tag="c_zr")
            nc.vector.memset(zr_s[:, :], 0.0)

            BB = 4
            for t in range(TPC):
                # indirect DMA derives the row coefficient from the source
                # SHAPE, so gather full 768-wide rows (cols 640:768 unused)
                gs_t = gp.tile([128, B_pad * 768], bf16, tag="gs")
                for b in range(B_pad):
                    col = t * 2 * PB + b
                    nc.gpsimd.indirect_dma_start(
                        out=gs_t[:, 768 * b:768 * (b + 1)], out_offset=None,
                        in_=T_full[:, :],
                        in_offset=bass.IndirectOffsetOnAxis(
                            ap=ixs_s[:, col:col + 1], axis=0))
                prt = cp.tile([128, 128], bf16, tag=f"c_tr{t}")
                nc.gpsimd.indirect_dma_start(
                    out=prt[:, :], out_offset=None,
                    in_=T_full[:, :].rearrange("r (a b) -> (r a) b", b=128),
                    in_offset=bass.IndirectOffsetOnAxis(
                        ap=trix_t[:, t:t + 1], axis=0))
                tl_q = wp.tile([9, EPT], i8, tag="tailq")
                nc.sync.dma_start(
                    out=tl_q[:, :],
                    in_=_blob_view(eblob, eoff["tail9"] + t * 9 * EPT,
                                   9 * EPT, 9, i8))
                tl_t = wp.tile([9, EPT], bf16, tag="tail")
                nc.scalar.activation(out=tl_t[:, :], in_=tl_q[:, :],
                                     func=AF.Copy,
                                     scale=scl_t[0:9, 2 * NT:2 * NT + 1])
                yh_q = wp.tile([128, 4 * B_pad], i8, tag="yh")
                nc.sync.dma_start(
                    out=yh_q[:, :],
                    in_=_blob_view(eblob, eoff["y4h"] + t * 128 * 4 * B_pad,
                                   128 * 4 * B_pad, 128, i8))
                yl_q = wp.tile([128, B_pad], u8, tag="yl")
                nc.sync.dma_start(
                    out=yl_q[:, :],
                    in_=_blob_view(eblob, eoff["y4l"] + t * 128 * B_pad,
                                   128 * B_pad, 128, u8))
                yl_u = wp.tile([128, 4 * B_pad], u8, tag="ylu")
                ylv = yl_u[:, :].rearrange("p (k four) -> p k four", four=4)
                nc.vector.tensor_scalar(out=ylv[:, :, 0], in0=yl_q[:, :],
                                        scalar1=3, scalar2=None,
                                        op0=AL.bitwise_and)
                for fi in range(1, 4):
                    nc.vector.tensor_scalar(out=ylv[:, :, fi], in0=yl_q[:, :],
                                            scalar1=2 * fi, scalar2=3,
                                            op0=AL.logical_shift_right,
                                            op1=AL.bitwise_and)
                y_t = wp.tile([128, 4 * B_pad], f32, tag="yrl")
                nc.scalar.activation(out=y_t[:, :], in_=yh_q[:, :],
                                     func=AF.Copy, scale=4.0)
                ylo_f = wp.tile([128, 4 * B_pad], f32, tag="ylf")
                nc.vector.tensor_copy(out=ylo_f[:, :], in_=yl_u[:, :])
                nc.vector.tensor_tensor(out=y_t[:, :], in0=y_t[:, :],
                                        in1=ylo_f[:, :], op=AL.add)
                nc.vector.tensor_scalar_add(out=y_t[:, :], in0=y_t[:, :],
                                            scalar1=-2.0)
                nc.scalar.activation(out=y_t[:, :], in_=y_t[:, :],
                                     func=AF.Copy,
                                     scale=scl_t[:, 2 * NT + 1:2 * NT + 2])
                rlq_t = wp.tile([128, B_pad], i8, tag="rlq")
                nc.sync.dma_start(
                    out=rlq_t[:, :],
                    in_=_blob_view(eblob, eoff["rl8"] + t * 128 * B_pad,
                                   128 * B_pad, 128, i8))
                rlb_t = wp.tile([128, B_pad], bf16, tag="rlb")
                nc.vector.tensor_copy(out=rlb_t[:, :], in_=rlq_t[:, :])

                sp_t = wp.tile([128, B_pad * 128], bf16, tag="spl")
                rl3 = rlb_t[:, :].unsqueeze(2)
                nc.vector.tensor_tensor(
                    out=sp_t[:, :].rearrange("p (b n) -> p b n", n=128),
                    in0=rl3.to_broadcast([128, B_pad, 128]),
                    in1=iotab_s[:, :].unsqueeze(1).to_broadcast(
                        [128, B_pad, 128]),
                    op=AL.is_equal)
                spf_t = wp.tile([128, B_pad * 128], f32, tag="spf")
                nc.vector.tensor_copy(out=spf_t[:, :], in_=sp_t[:, :])
                sy_t = wp.tile([128, B_pad * 384], bf16, tag="syl")
                y13 = (y_t[:, B_pad:4 * B_pad]
                       .rearrange("p (f b) -> p f b", f=3)
                       .transpose([0, 2, 1])
                       .unsqueeze(3))
                nc.gpsimd.tensor_tensor(
                    out=sy_t[:, :].rearrange("p (b f n) -> p b f n", f=3, n=128),
                    in0=sp_t[:, :].rearrange("p (b n) -> p b n", n=128)
                        .unsqueeze(2).to_broadcast([128, B_pad, 3, 128]),
                    in1=y13.to_broadcast([128, B_pad, 3, 128]),
                    op=AL.mult)

                st_t = gp.tile([128, EPT], bf16, tag="stT")
                for q in range(0, B_pad, 4):
                    qn = min(4, B_pad - q)
                    ptr = psB.tile([128, 512], f32, tag="pt1")
                    for i in range(qn):
                        nc.tensor.transpose(
                            out=ptr[:, 128 * i:128 * (i + 1)],
                            in_=spf_t[:, 128 * (q + i):128 * (q + i + 1)],
                            identity=idf_s[:, :])
                    nc.scalar.activation(out=st_t[:, 128 * q:128 * (q + qn)],
                                         in_=ptr[:, :128 * qn], func=AF.Copy)

                agg = psAgg.tile([128, 1024], f32, tag="agg")
                nc.tensor.matmul(out=agg[:, 0:512], lhsT=zr_s[:, :],
                                 rhs=w3_s, start=True, stop=False,
                                 skip_group_check=True)
                nc.tensor.matmul(out=agg[:, 512:1024], lhsT=zr_s[:, :],
                                 rhs=w3_s, start=True, stop=False,
                                 skip_group_check=True)

                nb_groups = (B_pad + BB - 1) // BB
                for g in range(nb_groups):
                    b0 = g * BB
                    gsz = min(BB, B_pad - b0)
                    p1 = psB.tile([128, 128 * BB], f32, tag="p1")
                    for bi in range(gsz):
                        b = b0 + bi
                        o = 128 * bi
                        nc.tensor.matmul(out=p1[:, o:o + 128],
                                         lhsT=tl_t[:, 128 * b:128 * (b + 1)],
                                         rhs=w1c_s, start=True, stop=False)
                        nc.tensor.matmul(out=p1[:, o:o + 128], lhsT=idb_s[:, :],
                                         rhs=gs_t[:, 768 * b + 512:768 * b + 640],
                                         start=False, stop=False)
                        nc.tensor.matmul(out=p1[:, o:o + 128],
                                         lhsT=st_t[:, 128 * b:128 * (b + 1)],
                                         rhs=prt[:, :],
                                         start=False, stop=False)
                        nc.tensor.matmul(out=p1[:, o:o + 128],
                                         lhsT=ones_s[:, :], rhs=b1_s,
                                         start=False, stop=True)
                    h1 = wp.tile([128, 128 * BB], f32, tag="h1")
                    nc.scalar.activation(out=h1[:, :128 * gsz],
                                         in_=p1[:, :128 * gsz], func=AF.Silu)
                    pt1 = psB.tile([128, 128 * BB], f32, tag="pt1")
                    for bi in range(gsz):
                        o = 128 * bi
                        nc.tensor.transpose(out=pt1[:, o:o + 128],
                                            in_=h1[:, o:o + 128],
                                            identity=idf_s[:, :])
                    h1t = wp.tile([128, 128 * BB], bf16, tag="h1t")
                    nc.scalar.activation(out=h1t[:, :128 * gsz],
                                         in_=pt1[:, :128 * gsz], func=AF.Copy)

                    p2 = psB.tile([128, 128 * BB], f32, tag="p2")
                    for bi in range(gsz):
                        o = 128 * bi
                        nc.tensor.matmul(out=p2[:, o:o + 128],
                                         lhsT=h1t[:, o:o + 128],
                                         rhs=w2_s, start=True, stop=False)
                        nc.tensor.matmul(out=p2[:, o:o + 128], lhsT=ones_s[:, :],
                                         rhs=b2_s, start=False, stop=True)
                    h2 = wp.tile([128, 128 * BB], f32, tag="h2")
                    nc.scalar.activation(out=h2[:, :128 * gsz],
                                         in_=p2[:, :128 * gsz], func=AF.Silu)
                    pt2 = psB.tile([128, 128 * BB], f32, tag="pt2")
                    for bi in range(gsz):
                        o = 128 * bi
                        nc.tensor.transpose(out=pt2[:, o:o + 128],
                                            in_=h2[:, o:o + 128],
                                            identity=idf_s[:, :])
                    h2t = wp.tile([128, 128 * BB], bf16, tag="h2t")
                    nc.scalar.activation(out=h2t[:, :128 * gsz],
                                         in_=pt2[:, :128 * gsz], func=AF.Copy)

                    for bi in range(gsz):
                        b = b0 + bi
                        o = 128 * bi
                        ptw = psC.tile([128, 512], f32, tag="ptw")
                        nc.tensor.matmul(out=ptw[:, :], lhsT=h2t[:, o:o + 128],
                                         rhs=w3_s, start=True, stop=True)
                        tpw = wp.tile([128, 512], bf16, tag="tpw")
                        nc.scalar.activation(out=tpw[:, :], in_=ptw[:, :],
                                             func=AF.Copy)

                        xs0 = gs_t[:, 768 * b:768 * b + 128]
                        xs1 = gs_t[:, 768 * b + 128:768 * b + 512]
                        y0 = y_t[:, b:b + 1]
                        pa = wp.tile([128, 128], bf16, tag="pa")
                        pd = wp.tile([128, 384], bf16, tag="pd")
                        pb = wp.tile([128, 128], bf16, tag="pb")
                        pc = wp.tile([128, 384], bf16, tag="pc")
                        nc.vector.tensor_tensor(out=pa[:, :], in0=xs0,
                                                in1=tpw[:, 0:128], op=AL.mult)
                        nc.scalar.activation(out=pa[:, :], in_=pa[:, :],
                                             func=AF.Copy, scale=y0)
                        wd3 = tpw[:, 128:256].unsqueeze(1).to_broadcast(
                            [128, 3, 128])
                        y13b = (y_t[:, B_pad + b:4 * B_pad:B_pad]
                                .unsqueeze(2)
                                .to_broadcast([128, 3, 128]))
                        nc.vector.tensor_tensor(
                            out=pd[:, :].rearrange("p (f n) -> p f n", f=3),
                            in0=xs1.rearrange("p (f n) -> p f n", f=3),
                            in1=wd3, op=AL.mult)
                        nc.vector.tensor_tensor(
                            out=pd[:, :].rearrange("p (f n) -> p f n", f=3),
                            in0=pd[:, :].rearrange("p (f n) -> p f n", f=3),
                            in1=y13b, op=AL.mult)
                        nc.vector.tensor_tensor(out=pb[:, :], in0=xs0,
                                                in1=tpw[:, 256:384], op=AL.mult)
                        wc3 = tpw[:, 384:512].unsqueeze(1).to_broadcast(
                            [128, 3, 128])
                        nc.vector.tensor_tensor(
                            out=pc[:, :].rearrange("p (f n) -> p f n", f=3),
                            in0=xs1.rearrange("p (f n) -> p f n", f=3),
                            in1=wc3, op=AL.mult)
                        nc.scalar.activation(out=pc[:, :], in_=pc[:, :],
                                             func=AF.Copy, scale=y0)

                        lastb = (b == B_pad - 1)
                        sp_b = sp_t[:, 128 * b:128 * (b + 1)]
                        nc.tensor.matmul(out=agg[:, 0:128], lhsT=pa[:, :],
                                         rhs=sp_b, start=False, stop=False,
                                         skip_group_check=True)
                        nc.tensor.matmul(out=agg[:, 128:512], lhsT=pb[:, :],
                                         rhs=sy_t[:, 384 * b:384 * (b + 1)],
                                         start=False, stop=lastb,
                                         skip_group_check=True)
                        for i in range(3):
                            nc.tensor.matmul(out=agg[:, 512:640],
                                             lhsT=pd[:, 128 * i:128 * (i + 1)],
                                             rhs=sp_b, start=False, stop=False,
                                             skip_group_check=True)
                        for i in range(3):
                            last = lastb and (i == 2)
                            nc.tensor.matmul(
                                out=agg[:, 640 + 128 * i:768 + 128 * i],
                                lhsT=pc[:, 128 * i:128 * (i + 1)],
                                rhs=sp_b, start=False, stop=last,
                                skip_group_check=True)

                # ---- final linear for this node tile ----
                aggs = wp.tile([128, 1024], bf16, tag="aggs")
                nc.scalar.activation(out=aggs[:, :], in_=agg[:, :],
                                     func=AF.Copy)
                pf = psC.tile([128, 512], f32, tag="ptw")
                nc.tensor.matmul(out=pf[:, 0:512], lhsT=zr_s[:, :],
                                 rhs=w3_s, start=True, stop=False,
                                 skip_group_check=True)
                nc.tensor.matmul(out=pf[:, 0:128], lhsT=aggs[:, 0:128],
                                 rhs=wo0t_s, start=False, stop=False,
                                 skip_group_check=True)
                nc.tensor.matmul(out=pf[:, 0:128], lhsT=aggs[:, 512:640],
                                 rhs=wo0b_s, start=False, stop=False,
                                 skip_group_check=True)
                for i in range(3):
                    o = 128 * (i + 1)
                    nc.tensor.matmul(out=pf[:, o:o + 128],
                                     lhsT=aggs[:, 128 + 128 * i:256 + 128 * i],
                                     rhs=wo1t_s, start=False, stop=False,
                                     skip_group_check=True)
                    nc.tensor.matmul(out=pf[:, o:o + 128],
                                     lhsT=aggs[:, 640 + 128 * i:768 + 128 * i],
                                     rhs=wo1b_s, start=False,
                                     stop=(i == 2), skip_group_check=True)
                mx = wp.tile([128, 1], f32, tag="mx")
                nc.vector.tensor_reduce(out=mx[:, :], in_=pf[:, 0:512],
                                        axis=mybir.AxisListType.XYZW,
                                        op=AL.max, apply_absolute_value=True)
                nc.vector.tensor_scalar_max(out=mx[:, :], in0=mx[:, :],
                                            scalar1=1e-20)
                sc = wp.tile([128, 1], f32, tag="sc")
                nc.vector.reciprocal(out=sc[:, :], in_=mx[:, :])
                nc.vector.tensor_scalar_mul(out=sc[:, :], in0=sc[:, :],
                                            scalar1=126.5)
                outs = wp.tile([128, 516], i8, tag="outs")
                ov = outs[:, 0:512].rearrange("p (m c) -> p m c", c=4)
                for c4 in range(4):
                    nc.scalar.activation(out=ov[:, :, c4],
                                         in_=pf[:, 128 * c4:128 * (c4 + 1)],
                                         func=AF.Copy, scale=sc[:, 0:1])
                nc.vector.tensor_copy(out=outs[:, 512:516].bitcast(f32),
                                      in_=sc[:, :])
                nc.sync.dma_start(out=out_d[128 * t:(t + 1) * 128, :],
                                  in_=outs[:, :])

    nc.compile()
    return nc


# --------------------------------------------------------------------------
# SPMD executor: node program + K chunk invocations, pipelined fetches.
# --------------------------------------------------------------------------

_exec_cache = {}
_fetch_pool = ThreadPoolExecutor(8)


def _make_fn(nc, mesh, spec):
    """Wrap a compiled Bass program in a cached shard_map'd jitted callable."""
    import jax
    try:
        from jax.experimental.shard_map import shard_map as _shard_map
    except ImportError:
        from jax import shard_map as _shard_map
    from concourse.bass2jax import _bass_exec_p, partition_id_tensor

    partition_name = (nc.partition_id_tensor.name
                      if nc.partition_id_tensor else None)
    in_names, out_names, out_avals = [], [], []
    for alloc in nc.m.functions[0].allocations:
        if not isinstance(alloc, mybir.MemoryLocationSet):
            continue
        name = alloc.memorylocations[0].name
        if alloc.kind == "ExternalInput":
            if name != partition_name:
                in_names.append(name)
        elif alloc.kind == "ExternalOutput":
            out_names.append(name)
            out_avals.append(jax.core.ShapedArray(
                tuple(alloc.tensor_shape), mybir.dt.np(alloc.dtype)))
    n_params = len(in_names)
    all_names = list(in_names) + list(out_names)
    if partition_name is not None:
        all_names.append(partition_name)

    def _body(*args):
        operands = list(args)
        if partition_name is not None:
            operands.append(partition_id_tensor())
        return tuple(_bass_exec_p.bind(
            *operands, out_avals=tuple(out_avals), in_names=tuple(all_names),
            out_names=tuple(out_names), lowering_input_output_aliases=(),
            sim_require_finite=True, sim_require_nnan=True, nc=nc))

    n_outs = len(out_names)
    fn = jax.jit(
        _shard_map(_body, mesh=mesh, in_specs=(spec,) * (n_params + n_outs),
                   out_specs=(spec,) * n_outs, check_rep=False),
        keep_unused=True)
    return fn, in_names, out_names, out_avals


def _get_exec(B_pad):
    key = (B_pad, K_CHUNKS)
    if key in _exec_cache:
        return _exec_cache[key]

    import jax
    try:
        jax.config.update("jax_compilation_cache_dir", "/tmp/jax_comp_cache")
        jax.config.update("jax_persistent_cache_min_compile_time_secs", 1.0)
    except Exception:
        pass
    from jax.sharding import Mesh, PartitionSpec, NamedSharding
    from concourse.bass2jax import install_neuronx_cc_hook

    install_neuronx_cc_hook()

    nc_node = _build_node(B_pad)
    nc_chunk = _build_chunk(B_pad)

    devices = jax.devices()[:NCORES]
    mesh = Mesh(np.asarray(devices), ("core",))
    spec = PartitionSpec("core")
    sh = NamedSharding(mesh, spec)

    fn_node, nin, nout, nav = _make_fn(nc_node, mesh, spec)
    fn_chunk, cin, cout, cav = _make_fn(nc_chunk, mesh, spec)

    def _dev_zeros(avals):
        outs = []
        for av in avals:
            gshape = (NCORES * av.shape[0], *av.shape[1:])
            z = jax.jit(
                lambda shape=gshape, dtype=av.dtype: jax.numpy.zeros(
                    shape, dtype),
                out_shardings=sh)()
            outs.append(z)
        return jax.block_until_ready(outs)

    st = {
        "fn_node": fn_node, "node_in": nin, "node_out": nout,
        "fn_chunk": fn_chunk, "chunk_in": cin, "chunk_out": cout,
        "z_node": _dev_zeros(nav), "z_chunk": _dev_zeros(cav),
        "sh": sh,
    }
    _exec_cache[key] = st
    return st


def _run_spmd(B_pad, blobs):
    """Upload blobs, run node + K chunk programs, fetch chunk outputs.

    Submission is interleaved (upload_k -> chunk_k -> fetch_k in a thread)
    so each chunk executes as soon as its edge blob lands and its fetch's
    ready-wait/request round trips overlap the remaining uploads.
    """
    import jax
    st = _get_exec(B_pad)
    sh = st["sh"]
    nb = jax.device_put(blobs["node"], sh)
    eb0 = jax.device_put(blobs["edges"][0], sh)
    node_outs = st["fn_node"](nb, *st["z_node"])
    named = dict(zip(st["node_out"], node_outs))
    chunk_args = [named[n] for n in st["chunk_in"][1:]]  # T_full, Wfull, ...
    futs = []
    out0 = st["fn_chunk"](eb0, *chunk_args, *st["z_chunk"])[0]
    futs.append(_fetch_pool.submit(np.asarray, out0))
    for k in range(1, K_CHUNKS):
        ebk = jax.device_put(blobs["edges"][k], sh)
        outk = st["fn_chunk"](ebk, *chunk_args, *st["z_chunk"])[0]
        futs.append(_fetch_pool.submit(np.asarray, outk))
    return [f.result() for f in futs]


def kernel(**inputs):
    B_pad, blobs, node_row = _host_prep(inputs)
    arrs = _run_spmd(B_pad, blobs)
    # arrs[k]: [NCORES*TPC*128, 516] int8; reassemble per-core tables
    full = np.concatenate(
        [a.reshape(NCORES, TPC * 128, 516) for a in arrs], axis=1
    ).reshape(NCORES * LPC, 516)
    q = full[:, 0:512].astype(np.float32)
    s = np.ascontiguousarray(full[:, 512:516]).view(np.float32)
    vals = q / s
    return vals[node_row].reshape(N, MUL, 4)


# revision 4
# speedup vs baseline: 1.0018x; 1.0018x over previous
"""Trainium2 Bass kernel for nn_DiffusionInteractionBlock (GNN message passing).

Strategy: shard EDGES by receiver node range across 8 cores (receiver-sharded
edge parallelism), exactly as the v1 kernel, but restructured around the axon
link profile (83 ms RTT, ~55 MB/s each way):

  * all host->device inputs are packed into ONE consolidated node blob plus K
    small per-chunk edge blobs (fewer, larger PJRT buffers -> higher effective
    upload bandwidth);
  * the device program is split into exe_node (node tables + AllGather) and a
    single compiled exe_chunk program invoked K times, each chunk producing
    1/K of the output, so chunk downloads overlap the remaining chunk uploads
    and each other (the link is full duplex);
  * all tile-position dependence in exe_chunk flows through data (gather
    indices shipped in the blob, receiver scalars fetched from T_full by
    indirect DMA), so one NEFF serves every chunk.

Numerics: 10-bit x0 / y, int8 x1 / edge tails, bf16 tables, int8 +
per-node-scale output (rel err ~1.18e-2, gate 2e-2).
"""

import os
import sys
from concurrent.futures import ThreadPoolExecutor

import numpy as np

sys.path.insert(0, "/opt/trn_rl_repo")

import ml_dtypes

from concourse import bacc, bass, mybir, tile

BF16 = ml_dtypes.bfloat16

N = 10000
E = 160000
MUL = 128
NCORES = 8
NPC = N // NCORES  # 1250 nodes per core
NT = 10            # node tiles of 128 per core (1280 >= 1250)
LPC = NT * 128     # padded local node count (1280)
NPAD = NCORES * LPC  # padded global table rows (10240)
SQ3 = float(np.sqrt(3.0))
INV = 1.0 / np.sqrt(MUL)
OUT_SCALE = 1.0 / (np.sqrt(2 * MUL) * 16.0)
K_CHUNKS = int(os.environ.get("K_CHUNKS", "10"))
TPC = NT // K_CHUNKS  # tiles per chunk

dt = mybir.dt


def _align(x, a=64):
    return (x + a - 1) // a * a


def _node_layout():
    off = {}
    o = 0
    o_ = lambda n, sz: (off.__setitem__(n, o), _align(o + sz))[1]
    o = o_("x0h", 128 * LPC)
    o = o_("x0l", 128 * (LPC // 4))
    o = o_("nf_q", 384 * LPC)
    o = o_("wcat", 128 * 208 * 2)
    o = o_("wsm", 14 * 128 * 2)
    o = o_("scl", 128 * 22 * 4)
    return off, o


def _edge_layout(B_pad):
    EPT = 128 * B_pad
    PB = (B_pad + 1) // 2  # packed idx col pairs per tile
    off = {}
    o = 0
    o_ = lambda n, sz: (off.__setitem__(n, o), _align(o + sz))[1]
    o = o_("idxp", 128 * TPC * PB * 4)
    o = o_("trix", 128 * TPC * 4)
    o = o_("tail9", TPC * 9 * EPT)
    o = o_("y4h", TPC * 128 * 4 * B_pad)
    o = o_("y4l", TPC * 128 * B_pad)
    o = o_("rl8", TPC * 128 * B_pad)
    return off, o, PB


# --------------------------------------------------------------------------
# Host-side preprocessing
# --------------------------------------------------------------------------

def _host_prep(inputs):
    import heapq

    node_feats = np.asarray(inputs["node_feats"], np.float32)
    edge_attrs = np.asarray(inputs["edge_attrs"], np.float32)
    edge_feats = np.asarray(inputs["edge_feats"], np.float32)
    lengths = np.asarray(inputs["lengths"], np.float32)
    edge_index = np.asarray(inputs["edge_index"], np.int64)
    W_scalar = np.asarray(inputs["W_scalar"], np.float32)
    W_up0 = np.asarray(inputs["W_up0"], np.float32)
    W_up1 = np.asarray(inputs["W_up1"], np.float32)
    W1 = np.asarray(inputs["W1"], np.float32)
    b1 = np.asarray(inputs["b1"], np.float32)
    W2 = np.asarray(inputs["W2"], np.float32)
    b2 = np.asarray(inputs["b2"], np.float32)
    W3 = np.asarray(inputs["W3"], np.float32)
    Wout0 = np.asarray(inputs["Wout0"], np.float32)
    Wout1 = np.asarray(inputs["Wout1"], np.float32)

    sender, receiver = edge_index[0], edge_index[1]

    # --- degree-balanced node -> (core, tile, pos) assignment
    G = NCORES * NT
    deg = np.bincount(receiver, minlength=N)
    node_bin = np.empty(N, np.int32)
    node_pos = np.empty(N, np.int32)
    bin_nodes = np.full((G, 128), -1, np.int64)
    heap = [(0, 0, g) for g in range(G)]
    for n in np.argsort(-deg, kind="stable"):
        while True:
            load, cnt, g = heapq.heappop(heap)
            if cnt < 128:
                break
        node_bin[n] = g
        node_pos[n] = cnt
        bin_nodes[g, cnt] = n
        heapq.heappush(heap, (load + int(deg[n]), cnt + 1, g))
    node_row = ((node_bin // NT) * LPC + (node_bin % NT) * 128
                + node_pos).astype(np.int32)

    gtile = node_bin[receiver]
    counts = np.bincount(gtile, minlength=G)
    B_pad = int(np.ceil(counts.max() / 128))
    EPT = 128 * B_pad

    order = np.argsort(gtile, kind="stable")
    epos = np.full((G, EPT), -1, np.int64)
    off = 0
    for g in range(G):
        c = counts[g]
        epos[g, :c] = order[off:off + c]
        off += c

    valid = epos >= 0
    ep = np.where(valid, epos, 0)
    es = np.where(valid, node_row[sender[ep]], 0).astype(np.int32)
    rl = np.where(valid, node_pos[receiver[ep]], 0)
    y = np.where(valid[..., None], edge_attrs[ep], 0.0)
    ef = np.where(valid[..., None], edge_feats[ep], 0.0)
    ln = np.where(valid[..., None], lengths[ep], 0.0)

    # idx [G, EPT] -> [NCORES, 128, NT, B_pad]; pack per-tile column pairs
    PB = (B_pad + 1) // 2
    idx_s = es.reshape(NCORES, NT, B_pad, 128)
    idx_s = np.transpose(idx_s, (0, 3, 1, 2))  # [NC, 128, NT, B_pad]
    if B_pad % 2:
        idx_s = np.concatenate(
            [idx_s, np.zeros((NCORES, 128, NT, 1), np.int32)], axis=3)
    idx_p = (idx_s[:, :, :, 0::2] | (idx_s[:, :, :, 1::2] << 16)).astype(
        np.int32)                                # [NC, 128, NT, PB]

    # tail9 [NCORES, NT, 9, EPT] int8 with per-(core,feature) scale
    tail = np.concatenate([ef, ln], axis=-1)
    tail9 = np.transpose(tail.reshape(NCORES, NT, EPT, 9), (0, 1, 3, 2))
    tmx = np.maximum(np.abs(tail9).max(axis=(1, 3)), 1e-20)
    tail9 = np.rint(tail9 * (126.5 / tmx)[:, None, :, None]).astype(np.int8)
    tail_s = (tmx / 126.5).astype(np.float32)

    # y as 10-bit fixed point (global per-core scale): q = 4*hi + (lo-2),
    # hi int8, lo 2 bits x4 per byte packed over the flat (comp, block) axis
    yv = y.reshape(NCORES, NT, B_pad, 128, 4)
    y4f = np.transpose(yv, (0, 1, 3, 4, 2))
    ymx = np.maximum(np.abs(y4f).max(axis=(1, 2, 3, 4)), 1e-20)
    qy = np.clip(np.rint(y4f * (509.0 / ymx)[:, None, None, None, None]),
                 -509, 509).astype(np.int32)
    yhi = np.floor_divide(qy + 2, 4)
    ylo = (qy - 4 * yhi + 2).reshape(NCORES, NT, 128, 4 * B_pad)
    y4h = yhi.astype(np.int8)
    y4l = (ylo[..., 0::4] | (ylo[..., 1::4] << 2) | (ylo[..., 2::4] << 4)
           | (ylo[..., 3::4] << 6)).astype(np.uint8)
    y_s = (ymx / 509.0).astype(np.float32)
    rl8 = rl.reshape(NCORES, NT, B_pad, 128).transpose(0, 1, 3, 2)
    rl8 = rl8.astype(np.int8)

    # per-core node features in bin-permuted column order
    x1 = node_feats[:, MUL:].reshape(N, MUL, 3)
    f = np.concatenate([node_feats[:, :MUL], x1[:, :, 0], x1[:, :, 1],
                        x1[:, :, 2]], axis=1)
    colnodes = bin_nodes.reshape(NCORES, LPC)
    vals = f[np.where(colnodes >= 0, colnodes, 0)]
    vals[colnodes < 0] = 0
    nfT = np.ascontiguousarray(vals.transpose(0, 2, 1))         # [NC, 512, LPC]
    x0T = nfT[:, 0:128]
    x1T = nfT[:, 128:512]
    nmx = np.maximum(np.abs(x1T).max(axis=1), 1e-20)
    nf_q = np.rint(x1T * (126.5 / nmx)[:, None, :]).astype(np.int8)
    nf_s = (nmx / 126.5).reshape(NCORES, NT, 128).transpose(0, 2, 1)
    # x0 as 10-bit fixed point with per-node scale: q = 4*hi + (lo-2),
    # hi int8, lo 2 bits x4 per byte
    mx0 = np.maximum(np.abs(x0T).max(axis=1), 1e-20)
    q0 = np.clip(np.rint(x0T * (509.0 / mx0)[:, None, :]),
                 -509, 509).astype(np.int32)
    hi = np.floor_divide(q0 + 2, 4)
    lo = q0 - 4 * hi + 2
    x0h = hi.astype(np.int8)
    x0l = (lo[:, :, 0::4] | (lo[:, :, 1::4] << 2) | (lo[:, :, 2::4] << 4)
           | (lo[:, :, 3::4] << 6)).astype(np.uint8)
    s0 = (mx0 / 509.0).reshape(NCORES, NT, 128).transpose(0, 2, 1)
    scl = np.zeros((NCORES, 128, 2 * NT + 2), np.float32)
    scl[:, :, :NT] = nf_s
    scl[:, :, NT:2 * NT] = s0
    scl[:, :9, 2 * NT] = tail_s
    scl[:, :, 2 * NT + 1] = y_s[:, None]

    Ws_inv = W_scalar * INV
    W3b = np.concatenate(
        [W3[:, :MUL], W3[:, MUL:2 * MUL] / SQ3,
         W3[:, 2 * MUL:3 * MUL], W3[:, 3 * MUL:]], axis=1)
    wcat = np.concatenate([
        W_up0 * INV, W_up1 * INV,
        Ws_inv @ W1[:MUL], Ws_inv @ W1[MUL:2 * MUL],
        W2, Wout0[:MUL] * OUT_SCALE, Wout0[MUL:] * OUT_SCALE,
        Wout1[:MUL] * OUT_SCALE, Wout1[MUL:] * OUT_SCALE, W3b,
    ], axis=1).astype(BF16)                                     # [128, 1664]
    wcat_sh = wcat.reshape(128, 8, 208).transpose(1, 0, 2).copy()  # [8,128,208]
    cw = np.stack([
        -2.0 * (W_up0 * INV).astype(BF16).astype(np.float32).sum(0),
        -2.0 * (Ws_inv @ W1[:MUL]).astype(BF16).astype(np.float32).sum(0),
        -2.0 * (Ws_inv @ W1[MUL:2 * MUL]).astype(BF16).astype(np.float32).sum(0),
    ])
    wsmall = np.concatenate(
        [W1[2 * MUL:], b1[None, :], b2[None, :], cw], 0).astype(BF16)  # [14,128]

    # ---- pack node blob [NCORES, NB] int8 ----
    noff, NB = _node_layout()
    nblob = np.zeros((NCORES, NB), np.int8)

    def put(o, arr):
        b = arr.reshape(NCORES, -1).view(np.int8)
        nblob[:, o:o + b.shape[1]] = b

    put(noff["x0h"], x0h)
    put(noff["x0l"], x0l)
    put(noff["nf_q"], nf_q)
    put(noff["wcat"], wcat_sh)
    wsm_all = np.tile(wsmall[None], (NCORES, 1, 1))
    put(noff["wsm"], wsm_all)
    put(noff["scl"], np.ascontiguousarray(scl))

    # ---- pack edge blobs: K arrays [NCORES, EB] int8 ----
    eoff, EB, _PB = _edge_layout(B_pad)
    eblobs = []
    core_base = (np.arange(NCORES, dtype=np.int32) * LPC)
    prow = np.arange(128, dtype=np.int32)
    for k in range(K_CHUNKS):
        t0 = k * TPC
        eb = np.zeros((NCORES, EB), np.int8)

        def pute(o, arr):
            b = arr.reshape(NCORES, -1).view(np.int8)
            eb[:, o:o + b.shape[1]] = b

        pute(eoff["idxp"], np.ascontiguousarray(idx_p[:, :, t0:t0 + TPC]))
        # T_full is gathered as a [NPAD*6, 128] view for the receiver-scalar
        # columns (indirect DMA requires source offset 0): row*6 + 5
        trix = (core_base[:, None, None]
                + (np.arange(t0, t0 + TPC, dtype=np.int32) * 128)[None, None, :]
                + prow[None, :, None]) * 6 + 5          # [NC, 128, TPC]
        pute(eoff["trix"], np.ascontiguousarray(trix))
        pute(eoff["tail9"], np.ascontiguousarray(tail9[:, t0:t0 + TPC]))
        pute(eoff["y4h"], np.ascontiguousarray(y4h[:, t0:t0 + TPC]))
        pute(eoff["y4l"], np.ascontiguousarray(y4l[:, t0:t0 + TPC]))
        pute(eoff["rl8"], np.ascontiguousarray(rl8[:, t0:t0 + TPC]))
        eblobs.append(eb)

    return B_pad, {"node": nblob, "edges": eblobs}, node_row


# --------------------------------------------------------------------------
# Device programs
# --------------------------------------------------------------------------

def _blob_view(blob, off, nbytes, p, dtype):
    """[p, nbytes/(p*isize)] view of a flat [1, NB] int8 dram tensor."""
    ap = blob[0:1, off:off + nbytes]
    if dtype != dt.int8:
        ap = ap.bitcast(dtype)
    n = ap.shape[1]
    return ap.rearrange("a (p c) -> (a p) c", p=p)


def _build_node(B_pad):
    noff, NB = _node_layout()
    nc = bacc.Bacc("TRN2", target_bir_lowering=False, debug=False,
                   num_devices=NCORES)
    f32, bf16, i8, u8 = dt.float32, dt.bfloat16, dt.int8, dt.uint8

    blob = nc.dram_tensor("nblob", [1, NB], i8, kind="ExternalInput")

    T_full = nc.dram_tensor("T_full", [NPAD, 768], bf16, kind="ExternalOutput")
    Wfull = nc.dram_tensor("Wfull", [128, 1664], bf16, kind="ExternalOutput")
    Wsm = nc.dram_tensor("Wsm", [14, 128], bf16, kind="ExternalOutput")
    sclf = nc.dram_tensor("sclf", [128, 22], f32, kind="ExternalOutput")

    T_loc = nc.dram_tensor("T_loc", [LPC, 768], bf16)
    T_gat = nc.dram_tensor("T_gat", [NPAD, 768], bf16)
    W_shb = nc.dram_tensor("W_shb", [128, 208], bf16)
    W_gat = nc.dram_tensor("W_gat", [NCORES * 128, 208], bf16)

    AL = mybir.AluOpType
    AF = mybir.ActivationFunctionType

    with tile.TileContext(nc) as tc:
        with (
            tc.tile_pool(name="const", bufs=1) as cp,
            tc.tile_pool(name="work", bufs=2) as wp,
            tc.tile_pool(name="psAgg", bufs=1, space="PSUM") as psAgg,
        ):
            # ---- allgather the column-sharded weight pack ----
            nc.sync.dma_start(
                out=W_shb[:, :],
                in_=_blob_view(blob, noff["wcat"], 128 * 208 * 2, 128, bf16))
            nc.gpsimd.collective_compute(
                "AllGather", mybir.AluOpType.bypass,
                replica_groups=[list(range(NCORES))],
                ins=[W_shb[:, :]], outs=[W_gat[:, :]])
            wc_t = cp.tile([128, 1664], bf16, tag="c_wcat")
            for k in range(NCORES):
                nc.sync.dma_start(
                    out=wc_t[:, 208 * k:208 * (k + 1)],
                    in_=W_gat[128 * k:128 * (k + 1), :])
            wsm_t = cp.tile([14, 128], bf16, tag="c_wsm")
            nc.sync.dma_start(
                out=wsm_t[:, :],
                in_=_blob_view(blob, noff["wsm"], 14 * 128 * 2, 14, bf16))
            cw_t = []
            for k in range(3):
                cwk = cp.tile([1, 128], bf16, tag=f"c_cw{k}")
                nc.sync.dma_start(
                    out=cwk[:, :],
                    in_=_blob_view(blob, noff["wsm"] + (11 + k) * 256,
                                   256, 1, bf16))
                cw_t.append(cwk)
            scl_t = cp.tile([128, 2 * NT + 2], f32, tag="c_scl")
            nc.sync.dma_start(
                out=scl_t[:, :],
                in_=_blob_view(blob, noff["scl"], 128 * 22 * 4, 128, f32))

            wup0_s = wc_t[:, 0:128]
            wup1_s = wc_t[:, 128:256]
            wps_s = wc_t[:, 256:384]
            wpr_s = wc_t[:, 384:512]
            ones_s = cp.tile([1, 128], bf16, tag="c_ones")
            nc.vector.memset(ones_s[:, :], 1.0)

            # write exported weight tensors for the chunk program
            nc.sync.dma_start(out=Wfull[:, :], in_=wc_t[:, :])
            nc.sync.dma_start(out=Wsm[:, :], in_=wsm_t[:, :])
            nc.sync.dma_start(out=sclf[:, :], in_=scl_t[:, :])

            # ---- local node-table phase ----
            with tc.tile_pool(name="nodes", bufs=1) as npool:
                xh_q = npool.tile([128, LPC], i8, tag="x0h")
                nc.sync.dma_start(
                    out=xh_q[:, :],
                    in_=_blob_view(blob, noff["x0h"], 128 * LPC, 128, i8))
                xl_q = npool.tile([128, LPC // 4], u8, tag="x0l")
                nc.sync.dma_start(
                    out=xl_q[:, :],
                    in_=_blob_view(blob, noff["x0l"], 128 * (LPC // 4), 128, u8))
                x0hi = npool.tile([128, LPC], bf16, tag="x0hi")
                nc.scalar.activation(out=x0hi[:, :], in_=xh_q[:, :],
                                     func=AF.Copy, scale=4.0)
                xl_u = npool.tile([128, LPC], u8, tag="x0lu")
                xlv = xl_u[:, :].rearrange("p (k four) -> p k four", four=4)
                nc.vector.tensor_scalar(out=xlv[:, :, 0], in0=xl_q[:, :],
                                        scalar1=3, scalar2=None,
                                        op0=AL.bitwise_and)
                for fi in range(1, 4):
                    nc.vector.tensor_scalar(out=xlv[:, :, fi], in0=xl_q[:, :],
                                            scalar1=2 * fi, scalar2=3,
                                            op0=AL.logical_shift_right,
                                            op1=AL.bitwise_and)
                x0lo = npool.tile([128, LPC], bf16, tag="x0lo")
                nc.vector.tensor_copy(out=x0lo[:, :], in_=xl_u[:, :])
                x1t0 = npool.tile([128, LPC], bf16, tag="nf1")
                x1t1 = npool.tile([128, LPC], bf16, tag="nf2")
                x1t2 = npool.tile([128, LPC], bf16, tag="nf3")
                for k, t in enumerate([x1t0, x1t1, x1t2]):
                    xq = npool.tile([128, LPC], i8, tag=f"nq{k}")
                    nc.sync.dma_start(
                        out=xq[:, :],
                        in_=_blob_view(blob, noff["nf_q"] + k * 128 * LPC,
                                       128 * LPC, 128, i8))
                    nc.vector.tensor_copy(out=t[:, :], in_=xq[:, :])
                for s in range(NT):
                    sl = slice(128 * s, 128 * (s + 1))
                    pn = psAgg.tile([128, 1024], f32, tag="agg")
                    for lhs, rhs, o in [(x1t0, wup1_s, 128),
                                        (x1t1, wup1_s, 256),
                                        (x1t2, wup1_s, 384)]:
                        nc.tensor.matmul(out=pn[:, o:o + 128], lhsT=lhs[:, sl],
                                         rhs=rhs, start=True, stop=True)
                    for k, (rhs, o) in enumerate([(wup0_s, 0), (wps_s, 512),
                                                  (wpr_s, 640)]):
                        nc.tensor.matmul(out=pn[:, o:o + 128],
                                         lhsT=x0hi[:, sl], rhs=rhs,
                                         start=True, stop=False)
                        nc.tensor.matmul(out=pn[:, o:o + 128],
                                         lhsT=x0lo[:, sl], rhs=rhs,
                                         start=False, stop=False)
                        nc.tensor.matmul(out=pn[:, o:o + 128],
                                         lhsT=ones_s[:, :], rhs=cw_t[k],
                                         start=False, stop=True)
                    tsb = wp.tile([128, 768], bf16, tag="tsb")
                    s0c = scl_t[:, NT + s:NT + s + 1]
                    nc.scalar.activation(out=tsb[:, 0:128], in_=pn[:, 0:128],
                                         func=AF.Copy, scale=s0c)
                    nc.scalar.activation(out=tsb[:, 128:512],
                                         in_=pn[:, 128:512],
                                         func=AF.Copy, scale=scl_t[:, s:s + 1])
                    nc.scalar.activation(out=tsb[:, 512:640],
                                         in_=pn[:, 512:640],
                                         func=AF.Copy, scale=s0c)
                    nc.scalar.activation(out=tsb[:, 640:768],
                                         in_=pn[:, 640:768],
                                         func=AF.Copy, scale=s0c)
                    nc.sync.dma_start(out=T_loc[sl, :], in_=tsb[:, :])

            # ---- allgather node tables (with receiver scalars) ----
            # (collectives cannot write IO tensors, so gather into internal
            # DRAM and copy out)
            nc.gpsimd.collective_compute(
                "AllGather", mybir.AluOpType.bypass,
                replica_groups=[list(range(NCORES))],
                ins=[T_loc[:, :]], outs=[T_gat[:, :]])
            nc.sync.dma_start(out=T_full[:, :], in_=T_gat[:, :])

    nc.compile()
    return nc


def _build_chunk(B_pad):
    eoff, EB, PB = _edge_layout(B_pad)
    EPT = 128 * B_pad
    nc = bacc.Bacc("TRN2", target_bir_lowering=False, debug=False,
                   num_devices=NCORES)
    f32, bf16, i32, i8, u8 = (dt.float32, dt.bfloat16, dt.int32,
                              dt.int8, dt.uint8)

    eblob = nc.dram_tensor("eblob", [1, EB], i8, kind="ExternalInput")
    T_full = nc.dram_tensor("T_full", [NPAD, 768], bf16, kind="ExternalInput")
    Wfull = nc.dram_tensor("Wfull", [128, 1664], bf16, kind="ExternalInput")
    Wsm = nc.dram_tensor("Wsm", [14, 128], bf16, kind="ExternalInput")
    sclf = nc.dram_tensor("sclf", [128, 22], f32, kind="ExternalInput")

    out_d = nc.dram_tensor("out_dram", [TPC * 128, 516], i8,
                           kind="ExternalOutput")

    AL = mybir.AluOpType
    AF = mybir.ActivationFunctionType

    with tile.TileContext(nc) as tc:
        with (
            tc.tile_pool(name="const", bufs=1) as cp,
            tc.tile_pool(name="work", bufs=2) as wp,
            tc.tile_pool(name="gath", bufs=2) as gp,
            tc.tile_pool(name="psB", bufs=1, space="PSUM") as psB,
            tc.tile_pool(name="psC", bufs=2, space="PSUM") as psC,
            tc.tile_pool(name="psAgg", bufs=1, space="PSUM") as psAgg,
        ):
            wc_t = cp.tile([128, 1664], bf16, tag="c_wcat")
            nc.sync.dma_start(out=wc_t[:, :], in_=Wfull[:, :])
            wsm_t = cp.tile([14, 128], bf16, tag="c_wsm")
            nc.sync.dma_start(out=wsm_t[:, :], in_=Wsm[:, :])
            b1_t = cp.tile([1, 128], bf16, tag="c_b1")
            nc.sync.dma_start(out=b1_t[:, :], in_=Wsm[9:10, :])
            b2_t = cp.tile([1, 128], bf16, tag="c_b2")
            nc.sync.dma_start(out=b2_t[:, :], in_=Wsm[10:11, :])
            scl_t = cp.tile([128, 2 * NT + 2], f32, tag="c_scl")
            nc.sync.dma_start(out=scl_t[:, :], in_=sclf[:, :])

            NBC = TPC * PB
            ixp_t = cp.tile([128, NBC], i32, tag="c_idxp")
            nc.sync.dma_start(
                out=ixp_t[:, :],
                in_=_blob_view(eblob, eoff["idxp"], 128 * NBC * 4, 128, i32))
            ixs_s = cp.tile([128, 2 * NBC], i32, tag="c_idx")
            ixv = ixs_s[:, :].rearrange("p (j two) -> p j two", two=2)
            nc.vector.tensor_scalar(out=ixv[:, :, 0], in0=ixp_t[:, :],
                                    scalar1=65535, scalar2=None,
                                    op0=AL.bitwise_and)
            nc.vector.tensor_scalar(out=ixv[:, :, 1], in0=ixp_t[:, :],
                                    scalar1=16, scalar2=None,
                                    op0=AL.logical_shift_right)
            trix_t = cp.tile([128, TPC], i32, tag="c_trix")
            nc.sync.dma_start(
                out=trix_t[:, :],
                in_=_blob_view(eblob, eoff["trix"], 128 * TPC * 4, 128, i32))

            w1c_s = wsm_t[0:9, :]
            b1_s = b1_t[:, :]
            b2_s = b2_t[:, :]
            w2_s = wc_t[:, 512:640]
            wo0t_s = wc_t[:, 640:768]
            wo0b_s = wc_t[:, 768:896]
            wo1t_s = wc_t[:, 896:1024]
            wo1b_s = wc_t[:, 1024:1152]
            w3_s = wc_t[:, 1152:1664]

            # iota / identity generated on device: col index (all partitions),
            # partition index (all cols), identity = is_equal of the two
            iotab_s = cp.tile([128, 128], bf16, tag="c_iotab")
            nc.gpsimd.iota(iotab_s[:, :], [[1, 128]], channel_multiplier=0,
                           allow_small_or_imprecise_dtypes=True)
            rowi_s = cp.tile([128, 128], bf16, tag="c_rowi")
            nc.gpsimd.iota(rowi_s[:, :], [[0, 128]], channel_multiplier=1,
                           allow_small_or_imprecise_dtypes=True)
            idb_s = cp.tile([128, 128], bf16, tag="c_idb")
            nc.vector.tensor_tensor(out=idb_s[:, :], in0=iotab_s[:, :],
                                    in1=rowi_s[:, :], op=AL.is_equal)
            idf_s = cp.tile([128, 128], f32, tag="c_idf")
            nc.vector.tensor_copy(out=idf_s[:, :], in_=idb_s[:, :])
            ones_s = cp.tile([1, 128], bf16, tag="c_ones")
            nc.vector.memset(ones_s[:, :], 1.0)
            zr_s = cp.tile([128, 128], bf16, tag="c_zr")
            nc.vector.memset(zr_s[:, :], 0.0)

            BB = 4
            for t in range(TPC):
                # indirect DMA derives the row coefficient from the source
                # SHAPE, so gather full 768-wide rows (cols 640:768 unused)
                gs_t = gp.tile([128, B_pad * 768], bf16, tag="gs")
                for b in range(B_pad):
                    col = t * 2 * PB + b
                    nc.gpsimd.indirect_dma_start(
                        out=gs_t[:, 768 * b:768 * (b + 1)], out_offset=None,
                        in_=T_full[:, :],
                        in_offset=bass.IndirectOffsetOnAxis(
                            ap=ixs_s[:, col:col + 1], axis=0))
                prt = cp.tile([128, 128], bf16, tag=f"c_tr{t}")
                nc.gpsimd.indirect_dma_start(
                    out=prt[:, :], out_offset=None,
                    in_=T_full[:, :].rearrange("r (a b) -> (r a) b", b=128),
                    in_offset=bass.IndirectOffsetOnAxis(
                        ap=trix_t[:, t:t + 1], axis=0))
                tl_q = wp.tile([9, EPT], i8, tag="tailq")
                nc.sync.dma_start(
                    out=tl_q[:, :],
                    in_=_blob_view(eblob, eoff["tail9"] + t * 9 * EPT,
                                   9 * EPT, 9, i8))
                tl_t = wp.tile([9, EPT], bf16, tag="tail")
                nc.scalar.activation(out=tl_t[:, :], in_=tl_q[:, :],
                                     func=AF.Copy,
                                     scale=scl_t[0:9, 2 * NT:2 * NT + 1])
                yh_q = wp.tile([128, 4 * B_pad], i8, tag="yh")
                nc.sync.dma_start(
                    out=yh_q[:, :],
                    in_=_blob_view(eblob, eoff["y4h"] + t * 128 * 4 * B_pad,
                                   128 * 4 * B_pad, 128, i8))
                yl_q = wp.tile([128, B_pad], u8, tag="yl")
                nc.sync.dma_start(
                    out=yl_q[:, :],
                    in_=_blob_view(eblob, eoff["y4l"] + t * 128 * B_pad,
                                   128 * B_pad, 128, u8))
                yl_u = wp.tile([128, 4 * B_pad], u8, tag="ylu")
                ylv = yl_u[:, :].rearrange("p (k four) -> p k four", four=4)
                nc.vector.tensor_scalar(out=ylv[:, :, 0], in0=yl_q[:, :],
                                        scalar1=3, scalar2=None,
                                        op0=AL.bitwise_and)
                for fi in range(1, 4):
                    nc.vector.tensor_scalar(out=ylv[:, :, fi], in0=yl_q[:, :],
                                            scalar1=2 * fi, scalar2=3,
                                            op0=AL.logical_shift_right,
                                            op1=AL.bitwise_and)
                y_t = wp.tile([128, 4 * B_pad], f32, tag="yrl")
                nc.scalar.activation(out=y_t[:, :], in_=yh_q[:, :],
                                     func=AF.Copy, scale=4.0)
                ylo_f = wp.tile([128, 4 * B_pad], f32, tag="ylf")
                nc.vector.tensor_copy(out=ylo_f[:, :], in_=yl_u[:, :])
                nc.vector.tensor_tensor(out=y_t[:, :], in0=y_t[:, :],
                                        in1=ylo_f[:, :], op=AL.add)
                nc.vector.tensor_scalar_add(out=y_t[:, :], in0=y_t[:, :],
                                            scalar1=-2.0)
                nc.scalar.activation(out=y_t[:, :], in_=y_t[:, :],
                                     func=AF.Copy,
                                     scale=scl_t[:, 2 * NT + 1:2 * NT + 2])
                rlq_t = wp.tile([128, B_pad], i8, tag="rlq")
                nc.sync.dma_start(
                    out=rlq_t[:, :],
                    in_=_blob_view(eblob, eoff["rl8"] + t * 128 * B_pad,
                                   128 * B_pad, 128, i8))
                rlb_t = wp.tile([128, B_pad], bf16, tag="rlb")
                nc.vector.tensor_copy(out=rlb_t[:, :], in_=rlq_t[:, :])

                sp_t = wp.tile([128, B_pad * 128], bf16, tag="spl")
                rl3 = rlb_t[:, :].unsqueeze(2)
                nc.vector.tensor_tensor(
                    out=sp_t[:, :].rearrange("p (b n) -> p b n", n=128),
                    in0=rl3.to_broadcast([128, B_pad, 128]),
                    in1=iotab_s[:, :].unsqueeze(1).to_broadcast(
                        [128, B_pad, 128]),
                    op=AL.is_equal)
                spf_t = wp.tile([128, B_pad * 128], f32, tag="spf")
                nc.vector.tensor_copy(out=spf_t[:, :], in_=sp_t[:, :])
                sy_t = wp.tile([128, B_pad * 384], bf16, tag="syl")
                y13 = (y_t[:, B_pad:4 * B_pad]
                       .rearrange("p (f b) -> p f b", f=3)
                       .transpose([0, 2, 1])
                       .unsqueeze(3))
                nc.gpsimd.tensor_tensor(
                    out=sy_t[:, :].rearrange("p (b f n) -> p b f n", f=3, n=128),
                    in0=sp_t[:, :].rearrange("p (b n) -> p b n", n=128)
                        .unsqueeze(2).to_broadcast([128, B_pad, 3, 128]),
                    in1=y13.to_broadcast([128, B_pad, 3, 128]),
                    op=AL.mult)

                st_t = gp.tile([128, EPT], bf16, tag="stT")
                for q in range(0, B_pad, 4):
                    qn = min(4, B_pad - q)
                    ptr = psB.tile([128, 512], f32, tag="pt1")
                    for i in range(qn):
                        nc.tensor.transpose(
                            out=ptr[:, 128 * i:128 * (i + 1)],
                            in_=spf_t[:, 128 * (q + i):128 * (q + i + 1)],
                            identity=idf_s[:, :])
                    nc.scalar.activation(out=st_t[:, 128 * q:128 * (q + qn)],
                                         in_=ptr[:, :128 * qn], func=AF.Copy)

                agg = psAgg.tile([128, 1024], f32, tag="agg")
                nc.tensor.matmul(out=agg[:, 0:512], lhsT=zr_s[:, :],
                                 rhs=w3_s, start=True, stop=False,
                                 skip_group_check=True)
                nc.tensor.matmul(out=agg[:, 512:1024], lhsT=zr_s[:, :],
                                 rhs=w3_s, start=True, stop=False,
                                 skip_group_check=True)

                nb_groups = (B_pad + BB - 1) // BB
                for g in range(nb_groups):
                    b0 = g * BB
                    gsz = min(BB, B_pad - b0)
                    p1 = psB.tile([128, 128 * BB], f32, tag="p1")
                    for bi in range(gsz):
                        b = b0 + bi
                        o = 128 * bi
                        nc.tensor.matmul(out=p1[:, o:o + 128],
                                         lhsT=tl_t[:, 128 * b:128 * (b + 1)],
                                         rhs=w1c_s, start=True, stop=False)
                        nc.tensor.matmul(out=p1[:, o:o + 128], lhsT=idb_s[:, :],
                                         rhs=gs_t[:, 768 * b + 512:768 * b + 640],
                                         start=False, stop=False)
                        nc.tensor.matmul(out=p1[:, o:o + 128],
                                         lhsT=st_t[:, 128 * b:128 * (b + 1)],
                                         rhs=prt[:, :],
                                         start=False, stop=False)
                        nc.tensor.matmul(out=p1[:, o:o + 128],
                                         lhsT=ones_s[:, :], rhs=b1_s,
                                         start=False, stop=True)
                    h1 = wp.tile([128, 128 * BB], f32, tag="h1")
                    nc.scalar.activation(out=h1[:, :128 * gsz],
                                         in_=p1[:, :128 * gsz], func=AF.Silu)
                    pt1 = psB.tile([128, 128 * BB], f32, tag="pt1")
                    for bi in range(gsz):
                        o = 128 * bi
                        nc.tensor.transpose(out=pt1[:, o:o + 128],
                                            in_=h1[:, o:o + 128],
                                            identity=idf_s[:, :])
                    h1t = wp.tile([128, 128 * BB], bf16, tag="h1t")
                    nc.scalar.activation(out=h1t[:, :128 * gsz],
                                         in_=pt1[:, :128 * gsz], func=AF.Copy)

                    p2 = psB.tile([128, 128 * BB], f32, tag="p2")
                    for bi in range(gsz):
                        o = 128 * bi
                        nc.tensor.matmul(out=p2[:, o:o + 128],
                                         lhsT=h1t[:, o:o + 128],
                                         rhs=w2_s, start=True, stop=False)
                        nc.tensor.matmul(out=p2[:, o:o + 128], lhsT=ones_s[:, :],
                                         rhs=b2_s, start=False, stop=True)
                    h2 = wp.tile([128, 128 * BB], f32, tag="h2")
                    nc.scalar.activation(out=h2[:, :128 * gsz],
                                         in_=p2[:, :128 * gsz], func=AF.Silu)
                    pt2 = psB.tile([128, 128 * BB], f32, tag="pt2")
                    for bi in range(gsz):
                        o = 128 * bi
                        nc.tensor.transpose(out=pt2[:, o:o + 128],
                                            in_=h2[:, o:o + 128],
                                            identity=idf_s[:, :])
                    h2t = wp.tile([128, 128 * BB], bf16, tag="h2t")
                    nc.scalar.activation(out=h2t[:, :128 * gsz],
                                         in_=pt2[:, :128 * gsz], func=AF.Copy)

                    for bi in range(gsz):
                        b = b0 + bi
                        o = 128 * bi
                        ptw = psC.tile([128, 512], f32, tag="ptw")
                        nc.tensor.matmul(out=ptw[:, :], lhsT=h2t[:, o:o + 128],
                                         rhs=w3_s, start=True, stop=True)
                        tpw = wp.tile([128, 512], bf16, tag="tpw")
                        nc.scalar.activation(out=tpw[:, :], in_=ptw[:, :],
                                             func=AF.Copy)

                        xs0 = gs_t[:, 768 * b:768 * b + 128]
                        xs1 = gs_t[:, 768 * b + 128:768 * b + 512]
                        y0 = y_t[:, b:b + 1]
                        pa = wp.tile([128, 128], bf16, tag="pa")
                        pd = wp.tile([128, 384], bf16, tag="pd")
                        pb = wp.tile([128, 128], bf16, tag="pb")
                        pc = wp.tile([128, 384], bf16, tag="pc")
                        nc.vector.tensor_tensor(out=pa[:, :], in0=xs0,
                                                in1=tpw[:, 0:128], op=AL.mult)
                        nc.scalar.activation(out=pa[:, :], in_=pa[:, :],
                                             func=AF.Copy, scale=y0)
                        wd3 = tpw[:, 128:256].unsqueeze(1).to_broadcast(
                            [128, 3, 128])
                        y13b = (y_t[:, B_pad + b:4 * B_pad:B_pad]
                                .unsqueeze(2)
                                .to_broadcast([128, 3, 128]))
                        nc.vector.tensor_tensor(
                            out=pd[:, :].rearrange("p (f n) -> p f n", f=3),
                            in0=xs1.rearrange("p (f n) -> p f n", f=3),
                            in1=wd3, op=AL.mult)
                        nc.vector.tensor_tensor(
                            out=pd[:, :].rearrange("p (f n) -> p f n", f=3),
                            in0=pd[:, :].rearrange("p (f n) -> p f n", f=3),
                            in1=y13b, op=AL.mult)
                        nc.vector.tensor_tensor(out=pb[:, :], in0=xs0,
                                                in1=tpw[:, 256:384], op=AL.mult)
                        wc3 = tpw[:, 384:512].unsqueeze(1).to_broadcast(
                            [128, 3, 128])
                        nc.vector.tensor_tensor(
                            out=pc[:, :].rearrange("p (f n) -> p f n", f=3),
                            in0=xs1.rearrange("p (f n) -> p f n", f=3),
                            in1=wc3, op=AL.mult)
                        nc.scalar.activation(out=pc[:, :], in_=pc[:, :],
                                             func=AF.Copy, scale=y0)

                        lastb = (b == B_pad - 1)
                        sp_b = sp_t[:, 128 * b:128 * (b + 1)]
                        nc.tensor.matmul(out=agg[:, 0:128], lhsT=pa[:, :],
                                         rhs=sp_b, start=False, stop=False,
                                         skip_group_check=True)
                        nc.tensor.matmul(out=agg[:, 128:512], lhsT=pb[:, :],
                                         rhs=sy_t[:, 384 * b:384 * (b + 1)],
                                         start=False, stop=lastb,
                                         skip_group_check=True)
                        for i in range(3):
                            nc.tensor.matmul(out=agg[:, 512:640],
                                             lhsT=pd[:, 128 * i:128 * (i + 1)],
                                             rhs=sp_b, start=False, stop=False,
                                             skip_group_check=True)
                        for i in range(3):
                            last = lastb and (i == 2)
                            nc.tensor.matmul(
                                out=agg[:, 640 + 128 * i:768 + 128 * i],
                                lhsT=pc[:, 128 * i:128 * (i + 1)],
                                rhs=sp_b, start=False, stop=last,
                                skip_group_check=True)

                # ---- final linear for this node tile ----
                aggs = wp.tile([128, 1024], bf16, tag="aggs")
                nc.scalar.activation(out=aggs[:, :], in_=agg[:, :],
                                     func=AF.Copy)
                pf = psC.tile([128, 512], f32, tag="ptw")
                nc.tensor.matmul(out=pf[:, 0:512], lhsT=zr_s[:, :],
                                 rhs=w3_s, start=True, stop=False,
                                 skip_group_check=True)
                nc.tensor.matmul(out=pf[:, 0:128], lhsT=aggs[:, 0:128],
                                 rhs=wo0t_s, start=False, stop=False,
                                 skip_group_check=True)
                nc.tensor.matmul(out=pf[:, 0:128], lhsT=aggs[:, 512:640],
                                 rhs=wo0b_s, start=False, stop=False,
                                 skip_group_check=True)
                for i in range(3):
                    o = 128 * (i + 1)
                    nc.tensor.matmul(out=pf[:, o:o + 128],
                                     lhsT=aggs[:, 128 + 128 * i:256 + 128 * i],
                                     rhs=wo1t_s, start=False, stop=False,
                                     skip_group_check=True)
                    nc.tensor.matmul(out=pf[:, o:o + 128],
                                     lhsT=aggs[:, 640 + 128 * i:768 + 128 * i],
                                     rhs=wo1b_s, start=False,
                                     stop=(i == 2), skip_group_check=True)
                mx = wp.tile([128, 1], f32, tag="mx")
                nc.vector.tensor_reduce(out=mx[:, :], in_=pf[:, 0:512],
                                        axis=mybir.AxisListType.XYZW,
                                        op=AL.max, apply_absolute_value=True)
                nc.vector.tensor_scalar_max(out=mx[:, :], in0=mx[:, :],
                                            scalar1=1e-20)
                sc = wp.tile([128, 1], f32, tag="sc")
                nc.vector.reciprocal(out=sc[:, :], in_=mx[:, :])
                nc.vector.tensor_scalar_mul(out=sc[:, :], in0=sc[:, :],
                                            scalar1=126.5)
                outs = wp.tile([128, 516], i8, tag="outs")
                ov = outs[:, 0:512].rearrange("p (m c) -> p m c", c=4)
                for c4 in range(4):
                    nc.scalar.activation(out=ov[:, :, c4],
                                         in_=pf[:, 128 * c4:128 * (c4 + 1)],
                                         func=AF.Copy, scale=sc[:, 0:1])
                nc.vector.tensor_copy(out=outs[:, 512:516].bitcast(f32),
                                      in_=sc[:, :])
                nc.sync.dma_start(out=out_d[128 * t:(t + 1) * 128, :],
                                  in_=outs[:, :])

    nc.compile()
    return nc


# --------------------------------------------------------------------------
# SPMD executor: node program + K chunk invocations, pipelined fetches.
# --------------------------------------------------------------------------

_exec_cache = {}
_fetch_pool = ThreadPoolExecutor(16)


def _make_fn(nc, mesh, spec):
    """Wrap a compiled Bass program in a cached shard_map'd jitted callable."""
    import jax
    try:
        from jax.experimental.shard_map import shard_map as _shard_map
    except ImportError:
        from jax import shard_map as _shard_map
    from concourse.bass2jax import _bass_exec_p, partition_id_tensor

    partition_name = (nc.partition_id_tensor.name
                      if nc.partition_id_tensor else None)
    in_names, out_names, out_avals = [], [], []
    for alloc in nc.m.functions[0].allocations:
        if not isinstance(alloc, mybir.MemoryLocationSet):
            continue
        name = alloc.memorylocations[0].name
        if alloc.kind == "ExternalInput":
            if name != partition_name:
                in_names.append(name)
        elif alloc.kind == "ExternalOutput":
            out_names.append(name)
            out_avals.append(jax.core.ShapedArray(
                tuple(alloc.tensor_shape), mybir.dt.np(alloc.dtype)))
    n_params = len(in_names)
    all_names = list(in_names) + list(out_names)
    if partition_name is not None:
        all_names.append(partition_name)

    def _body(*args):
        operands = list(args)
        if partition_name is not None:
            operands.append(partition_id_tensor())
        return tuple(_bass_exec_p.bind(
            *operands, out_avals=tuple(out_avals), in_names=tuple(all_names),
            out_names=tuple(out_names), lowering_input_output_aliases=(),
            sim_require_finite=True, sim_require_nnan=True, nc=nc))

    n_outs = len(out_names)
    fn = jax.jit(
        _shard_map(_body, mesh=mesh, in_specs=(spec,) * (n_params + n_outs),
                   out_specs=(spec,) * n_outs, check_rep=False),
        keep_unused=True)
    return fn, in_names, out_names, out_avals


def _get_exec(B_pad):
    key = (B_pad, K_CHUNKS)
    if key in _exec_cache:
        return _exec_cache[key]

    import jax
    try:
        jax.config.update("jax_compilation_cache_dir", "/tmp/jax_comp_cache")
        jax.config.update("jax_persistent_cache_min_compile_time_secs", 1.0)
    except Exception:
        pass
    from jax.sharding import Mesh, PartitionSpec, NamedSharding
    from concourse.bass2jax import install_neuronx_cc_hook

    install_neuronx_cc_hook()

    nc_node = _build_node(B_pad)
    nc_chunk = _build_chunk(B_pad)

    devices = jax.devices()[:NCORES]
    mesh = Mesh(np.asarray(devices), ("core",))
    spec = PartitionSpec("core")
    sh = NamedSharding(mesh, spec)

    fn_node, nin, nout, nav = _make_fn(nc_node, mesh, spec)
    fn_chunk, cin, cout, cav = _make_fn(nc_chunk, mesh, spec)

    def _dev_zeros(avals):
        outs = []
        for av in avals:
            gshape = (NCORES * av.shape[0], *av.shape[1:])
            z = jax.jit(
                lambda shape=gshape, dtype=av.dtype: jax.numpy.zeros(
                    shape, dtype),
                out_shardings=sh)()
            outs.append(z)
        return jax.block_until_ready(outs)

    st = {
        "fn_node": fn_node, "node_in": nin, "node_out": nout,
        "fn_chunk": fn_chunk, "chunk_in": cin, "chunk_out": cout,
        "z_node": _dev_zeros(nav), "z_chunk": _dev_zeros(cav),
        "sh": sh,
    }
    _exec_cache[key] = st
    return st


def _run_spmd(B_pad, blobs):
    """Upload blobs, run node + K chunk programs, fetch chunk outputs.

    Submission is interleaved (upload_k -> chunk_k -> fetch_k in a thread)
    so each chunk executes as soon as its edge blob lands and its fetch's
    ready-wait/request round trips overlap the remaining uploads.
    """
    import jax
    st = _get_exec(B_pad)
    sh = st["sh"]
    nb = jax.device_put(blobs["node"], sh)
    node_outs = st["fn_node"](nb, *st["z_node"])
    eb0 = jax.device_put(blobs["edges"][0], sh)
    named = dict(zip(st["node_out"], node_outs))
    chunk_args = [named[n] for n in st["chunk_in"][1:]]  # T_full, Wfull, ...
    futs = []
    out0 = st["fn_chunk"](eb0, *chunk_args, *st["z_chunk"])[0]
    futs.append(_fetch_pool.submit(np.asarray, out0))
    for k in range(1, K_CHUNKS):
        ebk = jax.device_put(blobs["edges"][k], sh)
        outk = st["fn_chunk"](ebk, *chunk_args, *st["z_chunk"])[0]
        futs.append(_fetch_pool.submit(np.asarray, outk))
    return [f.result() for f in futs]


def kernel(**inputs):
    B_pad, blobs, node_row = _host_prep(inputs)
    arrs = _run_spmd(B_pad, blobs)
    # arrs[k]: [NCORES*TPC*128, 516] int8; reassemble per-core tables
    full = np.concatenate(
        [a.reshape(NCORES, TPC * 128, 516) for a in arrs], axis=1
    ).reshape(NCORES * LPC, 516)
    q = full[:, 0:512].astype(np.float32)
    s = np.ascontiguousarray(full[:, 512:516]).view(np.float32)
    vals = q / s
    return vals[node_row].reshape(N, MUL, 4)
